# revision 11
# baseline (speedup 1.0000x reference)
"""Trainium2 Bass kernel for the DMamba block (selective state-space / Mamba).

Sharding: tensor-parallel over d_inner across 8 NeuronCores (256 channels
each).  Everything on-chip is kept d-major ([d on partitions, token on free])
so the selective scan maps onto the native DVE tensor_tensor_scan
instruction (one recurrence per partition along the free/time axis).

Pipeline per core (d_local = 256 = 2 partition-tiles):
  in_proj GEMM (PE, f32r) -> causal conv via 4 diag-matmuls (PE) + silu ->
  x_proj partial GEMM -> AllReduce(x_dbl) -> dt_proj GEMM + softplus ->
  per (b, n): dA = exp(A*delta) (ACT, per-partition scale),
              dBu = dx * B (DVE; B row broadcast via DMA),
              h = tensor_tensor_scan(dA, dBu) (DVE),
              p = h * C (GPSIMD),
              y += p (PE identity-matmul accumulation in PSUM)
  -> gating y*silu(res) -> out_proj GEMM -> ReduceScatter -> host concat.
"""

import os
import sys
import time
from contextlib import ExitStack

import numpy as np

for _p in ("/opt/trn_rl_repo", "/root/.axon_site/_ro/trn_rl_repo"):
    if os.path.isdir(_p) and _p not in sys.path:
        sys.path.append(_p)

import concourse.bacc as bacc
import concourse.mybir as mybir
import concourse.tile as tile
from concourse.bass_utils import run_bass_kernel_spmd

F32 = mybir.dt.float32
F32R = mybir.dt.float32r
AF = mybir.ActivationFunctionType
OP = mybir.AluOpType

CFG_FULL = dict(B=2, L=1024, DM=1024, DI=2048, NST=16, RK=64, K4=4, NC=8)


def build_nc(cfg):
    B, L, DM, DI = cfg["B"], cfg["L"], cfg["DM"], cfg["DI"]
    NST, RK, K4, NC = cfg["NST"], cfg["RK"], cfg["K4"], cfg["NC"]
    DL = DI // NC                 # local d_inner channels
    NDT = DL // 128               # local partition-tiles of d
    TOK = B * L
    CH = min(512, L)              # psum free-dim chunk
    LCH = L // CH                 # chunks per sequence
    TCH = TOK // CH               # chunks per full token axis
    KC = DM // 128                # contraction chunks over d_model
    OCH = min(CH, DM)             # out_proj free chunk size
    DMCH = DM // OCH              # out_proj free chunks
    XR = RK + 2 * NST             # x_dbl rows

    nc = bacc.Bacc("TRN2", target_bir_lowering=False, debug=False, num_devices=NC)

    def din(name, shape, dt=F32):
        return nc.dram_tensor(name, shape, dt, kind="ExternalInput").ap()

    xT = din("xT", [DM, TOK], F32R)
    w_in_T = din("w_in_T", [DM, 2 * DL], F32R)
    convdiag = din("convdiag", [NDT * K4 * 128, 128], F32R)
    w_x_T = din("w_x_T", [DL, XR], F32R)
    w_dt_T = din("w_dt_T", [RK, DL], F32R)
    b_dt_col = din("b_dt_col", [DL, 1])
    a_col = din("a_col", [DL, NST])
    d_col = din("d_col", [DL, 1])
    conv_b_col = din("conv_b_col", [DL, 1])
    w_out_T = din("w_out_T", [DL, DM], F32R)
    id128 = din("id128", [128, 128], F32R)

    out_ext = nc.dram_tensor("out", [TOK // NC, DM], F32, kind="ExternalOutput").ap()

    shared = "Shared" if NC > 4 else "Local"
    xdbl_part = nc.dram_tensor("xdbl_part", [XR, TOK], F32).ap()
    xdbl_full = nc.dram_tensor("xdbl_full", [XR, TOK], F32, addr_space=shared).ap()
    out_part = nc.dram_tensor("out_part", [TOK, DM], F32).ap()
    out_rs = nc.dram_tensor("out_rs", [TOK // NC, DM], F32).ap()

    groups = [list(range(NC))]

    with tile.TileContext(nc) as tc, ExitStack() as ctx:
        consts = ctx.enter_context(tc.tile_pool(name="consts", bufs=1))
        big = ctx.enter_context(tc.tile_pool(name="big", bufs=1))
        work = ctx.enter_context(tc.tile_pool(name="work", bufs=2))
        mm = ctx.enter_context(tc.tile_pool(name="mm", bufs=4, space="PSUM"))

        # ---- constants ----
        a_t, d_t, bdt_t, cb_t, wout_t, wx_t, cdg_t = [], [], [], [], [], [], []
        for dt in range(NDT):
            t = consts.tile([128, NST], F32, name=f"a{dt}")
            nc.sync.dma_start(out=t[:], in_=a_col[dt * 128:(dt + 1) * 128, :])
            a_t.append(t)
            t = consts.tile([128, 1], F32, name=f"d{dt}")
            nc.sync.dma_start(out=t[:], in_=d_col[dt * 128:(dt + 1) * 128, :])
            d_t.append(t)
            t = consts.tile([128, 1], F32, name=f"bdt{dt}")
            nc.sync.dma_start(out=t[:], in_=b_dt_col[dt * 128:(dt + 1) * 128, :])
            bdt_t.append(t)
            t = consts.tile([128, 1], F32, name=f"cb{dt}")
            nc.sync.dma_start(out=t[:], in_=conv_b_col[dt * 128:(dt + 1) * 128, :])
            cb_t.append(t)
            t = consts.tile([128, DM], F32R, name=f"wo{dt}")
            nc.scalar.dma_start(out=t[:], in_=w_out_T[dt * 128:(dt + 1) * 128, :])
            wout_t.append(t)
            t = consts.tile([128, XR], F32R, name=f"wx{dt}")
            nc.sync.dma_start(out=t[:], in_=w_x_T[dt * 128:(dt + 1) * 128, :])
            wx_t.append(t)
            row = []
            for i in range(K4):
                t = consts.tile([128, 128], F32R, name=f"cd{dt}_{i}")
                off = (dt * K4 + i) * 128
                nc.sync.dma_start(out=t[:], in_=convdiag[off:off + 128, :])
                row.append(t)
            cdg_t.append(row)
        wdt_t = consts.tile([RK, DL], F32R, name="wdt")
        nc.sync.dma_start(out=wdt_t[:], in_=w_dt_T[:])
        id_t = consts.tile([128, 128], F32R, name="id128")
        nc.sync.dma_start(out=id_t[:], in_=id128[:])

        # ---- persistent intermediates (live through the scan) ----
        xc_t = [big.tile([128, TOK], F32R, name=f"xc{dt}") for dt in range(NDT)]
        sres_t = [big.tile([128, TOK], F32, name=f"sres{dt}") for dt in range(NDT)]
        delta_t = [big.tile([128, TOK], F32, name=f"delta{dt}") for dt in range(NDT)]
        dx_t = [big.tile([128, TOK], F32, name=f"dx{dt}") for dt in range(NDT)]

        with tc.tile_pool(name="early", bufs=1) as early:
            xz_pad = [[early.tile([128, L + K4 - 1], F32R, name=f"xzp{b}_{dt}")
                       for dt in range(NDT)] for b in range(B)]
            xdbl_sb = early.tile([XR, TOK], F32, name="xdbl_sb")
            for b in range(B):
                for dt in range(NDT):
                    nc.gpsimd.memset(xz_pad[b][dt][:, 0:K4 - 1].bitcast(F32), 0.0)

            # ---- in_proj: xz/res = W_in_sel @ x  (d-major out) ----
            with tc.tile_pool(name="xtp", bufs=1) as xtp:
                win_t = []
                for k in range(KC):
                    t = xtp.tile([128, 2 * DL], F32R, name=f"win{k}")
                    nc.scalar.dma_start(out=t[:], in_=w_in_T[k * 128:(k + 1) * 128, :])
                    win_t.append(t)
                for tch in range(TCH):
                    xtc = []
                    for k in range(KC):
                        t = xtp.tile([128, CH], F32R, name=f"xtc{k}", bufs=2)
                        nc.sync.dma_start(
                            out=t[:],
                            in_=xT[k * 128:(k + 1) * 128, tch * CH:(tch + 1) * CH])
                        xtc.append(t)
                    b = (tch * CH) // L
                    off = (tch * CH) % L
                    for m in range(2 * NDT):
                        ps = mm.tile([128, CH], F32, name="ps")
                        for k in range(KC):
                            nc.tensor.matmul(
                                ps[:], win_t[k][:, m * 128:(m + 1) * 128],
                                xtc[k][:],
                                start=(k == 0), stop=(k == KC - 1))
                        if m < NDT:
                            nc.scalar.activation(
                                xz_pad[b][m][:, K4 - 1 + off:K4 - 1 + off + CH],
                                ps[:], AF.Copy)
                        else:
                            # res chunk: sres = res * sigmoid(res) on the fly
                            dt = m - NDT
                            rtmp = work.tile([128, CH], F32, name="rtmp")
                            nc.scalar.activation(rtmp[:], ps[:], AF.Copy)
                            rsg = work.tile([128, CH], F32, name="rsg")
                            nc.scalar.activation(rsg[:], ps[:], AF.Sigmoid)
                            nc.gpsimd.tensor_tensor(
                                sres_t[dt][:, tch * CH:(tch + 1) * CH],
                                rtmp[:], rsg[:], OP.mult)

            # ---- causal depthwise conv (4 taps as diag matmuls) + silu ----
            for b in range(B):
                for dt in range(NDT):
                    for lc in range(LCH):
                        ps = mm.tile([128, CH], F32, name="ps")
                        for i in range(K4):
                            nc.tensor.matmul(
                                ps[:], cdg_t[dt][i],
                                xz_pad[b][dt][:, lc * CH + i:lc * CH + i + CH],
                                start=(i == 0), stop=(i == K4 - 1))
                        sg = work.tile([128, CH], F32, name="csg")
                        nc.scalar.activation(sg[:], ps[:], AF.Sigmoid,
                                             bias=cb_t[dt][:])
                        xcp = work.tile([128, CH], F32, name="cxp")
                        nc.scalar.activation(xcp[:], ps[:], AF.Identity,
                                             bias=cb_t[dt][:])
                        nc.gpsimd.tensor_tensor(
                            xc_t[dt][:, b * L + lc * CH:b * L + (lc + 1) * CH],
                            xcp[:], sg[:], OP.mult)

            # ---- x_dbl partial = W_x_sel @ xc ; AllReduce ----
            for tch in range(TCH):
                ps = mm.tile([XR, CH], F32, name="ps")
                for dt in range(NDT):
                    nc.tensor.matmul(
                        ps[:], wx_t[dt][:],
                        xc_t[dt][:, tch * CH:(tch + 1) * CH],
                        start=(dt == 0), stop=(dt == NDT - 1))
                nc.scalar.activation(xdbl_sb[:, tch * CH:(tch + 1) * CH], ps[:],
                                     AF.Copy)
            nc.sync.dma_start(out=xdbl_part[:], in_=xdbl_sb[:])

        nc.gpsimd.collective_compute(
            "AllReduce", OP.add, replica_groups=groups,
            ins=[xdbl_part[:]], outs=[xdbl_full[:]])

        # ---- delta = softplus(W_dt_sel @ dlt + b_dt)  (exp then ln(1+e)) ----
        for tch in range(TCH):
            dltc = work.tile([RK, CH], F32, name="dltc")
            nc.sync.dma_start(out=dltc[:],
                              in_=xdbl_full[0:RK, tch * CH:(tch + 1) * CH])
            dltr = work.tile([RK, CH], F32R, name="dltr")
            nc.vector.tensor_copy(dltr[:], dltc[:])
            for dt in range(NDT):
                ps = mm.tile([128, CH], F32, name="ps")
                nc.tensor.matmul(
                    ps[:], wdt_t[:, dt * 128:(dt + 1) * 128], dltr[:],
                    start=True, stop=True)
                e = work.tile([128, CH], F32, name="de")
                nc.scalar.activation(e[:], ps[:], AF.Exp, bias=bdt_t[dt][:])
                nc.scalar.activation(
                    delta_t[dt][:, tch * CH:(tch + 1) * CH], e[:], AF.Ln, bias=1.0)

        # ---- dx = delta * xc ----
        for dt in range(NDT):
            nc.vector.tensor_tensor(
                dx_t[dt][:], delta_t[dt][:], xc_t[dt][:].bitcast(F32), OP.mult)

        # ---- selective scan over (b, n) ----
        bc = ctx.enter_context(tc.tile_pool(name="bc", bufs=2))
        scanp = ctx.enter_context(tc.tile_pool(name="scanp", bufs=2))
        acc = ctx.enter_context(tc.tile_pool(name="acc", bufs=1, space="PSUM"))
        for b in range(B):
            y_ps = [acc.tile([128, L], F32, name=f"y{dt}") for dt in range(NDT)]
            for n in range(NST):
                brep = bc.tile([128, L], F32, name="brep")
                nc.sync.dma_start(
                    out=brep[:],
                    in_=xdbl_full[RK + n:RK + n + 1,
                                  b * L:(b + 1) * L].to_broadcast((128, L)))
                crep = bc.tile([128, L], F32, name="crep")
                nc.scalar.dma_start(
                    out=crep[:],
                    in_=xdbl_full[RK + NST + n:RK + NST + n + 1,
                                  b * L:(b + 1) * L].to_broadcast((128, L)))
                for dt in range(NDT):
                    dA = scanp.tile([128, L], F32, name="dA")
                    nc.scalar.activation(
                        dA[:], delta_t[dt][:, b * L:(b + 1) * L], AF.Exp,
                        scale=a_t[dt][:, n:n + 1])
                    dBu = scanp.tile([128, L], F32, name="dBu")
                    nc.vector.tensor_tensor(
                        dBu[:], dx_t[dt][:, b * L:(b + 1) * L], brep[:], OP.mult)
                    h = scanp.tile([128, L], F32, name="h")
                    nc.vector.tensor_tensor_scan(
                        h[:], dA[:], dBu[:], 0.0, OP.mult, OP.add)
                    p = scanp.tile([128, L], F32R, name="p")
                    nc.gpsimd.tensor_tensor(p[:], h[:], crep[:], OP.mult)
                    for lc in range(LCH):
                        nc.tensor.matmul(
                            y_ps[dt][:, lc * CH:(lc + 1) * CH], id_t[:],
                            p[:, lc * CH:(lc + 1) * CH],
                            start=(n == 0), stop=(n == NST - 1),
                            skip_group_check=True)
            # gating: yg = (y + D*xc) * sres   (reads y from PSUM directly)
            yg = []
            for dt in range(NDT):
                t1 = work.tile([128, L], F32, name="t1")
                nc.vector.scalar_tensor_tensor(
                    t1[:], xc_t[dt][:, b * L:(b + 1) * L].bitcast(F32),
                    d_t[dt][:], y_ps[dt][:], OP.mult, OP.add)
                ygt = scanp.tile([128, L], F32R, name=f"yg{dt}")
                nc.vector.tensor_tensor(
                    ygt[:], t1[:], sres_t[dt][:, b * L:(b + 1) * L], OP.mult)
                yg.append(ygt)
            # out_proj for this batch
            for m in range(L // 128):
                ot = work.tile([128, DM], F32, name="osb")
                for nch in range(DMCH):
                    ps = mm.tile([128, OCH], F32, name="ps")
                    for dt in range(NDT):
                        nc.tensor.matmul(
                            ps[:], yg[dt][:, m * 128:(m + 1) * 128],
                            wout_t[dt][:, nch * OCH:(nch + 1) * OCH],
                            start=(dt == 0), stop=(dt == NDT - 1))
                    nc.scalar.activation(ot[:, nch * OCH:(nch + 1) * OCH], ps[:],
                                         AF.Copy)
                nc.sync.dma_start(
                    out=out_part[b * L + m * 128:b * L + (m + 1) * 128, :],
                    in_=ot[:])

        nc.gpsimd.collective_compute(
            "ReduceScatter", OP.add, replica_groups=groups,
            ins=[out_part[:]], outs=[out_rs[:]])
        nc.sync.dma_start(out=out_ext[:], in_=out_rs[:])

    nc.compile()
    return nc


def prep_inputs(inputs, cfg):
    """Host-side sharding/transposition. Returns per-core input maps."""
    B, L, DM, DI = cfg["B"], cfg["L"], cfg["DM"], cfg["DI"]
    NST, RK, K4, NC = cfg["NST"], cfg["RK"], cfg["K4"], cfg["NC"]
    DL = DI // NC
    NDT = DL // 128
    TOK = B * L

    x = np.asarray(inputs["x"], np.float32)
    W_in = np.asarray(inputs["W_in"], np.float32)
    conv_w = np.asarray(inputs["conv_w"], np.float32)
    conv_b = np.asarray(inputs["conv_b"], np.float32)
    W_x = np.asarray(inputs["W_x"], np.float32)
    W_dt = np.asarray(inputs["W_dt"], np.float32)
    b_dt = np.asarray(inputs["b_dt"], np.float32)
    A_log = np.asarray(inputs["A_log"], np.float32)
    D = np.asarray(inputs["D"], np.float32)
    W_out = np.asarray(inputs["W_out"], np.float32)

    xT = np.ascontiguousarray(x.reshape(TOK, DM).T)
    id128 = np.eye(128, dtype=np.float32)
    in_maps = []
    for c in range(NC):
        sl = slice(c * DL, (c + 1) * DL)
        w_in_sel = np.concatenate([W_in[sl], W_in[DI + c * DL:DI + (c + 1) * DL]], 0)
        cd = np.zeros((NDT * K4 * 128, 128), np.float32)
        for dt in range(NDT):
            for i in range(K4):
                off = (dt * K4 + i) * 128
                np.fill_diagonal(cd[off:off + 128],
                                 conv_w[c * DL + dt * 128:c * DL + (dt + 1) * 128, i])
        in_maps.append({
            "xT": xT,
            "w_in_T": np.ascontiguousarray(w_in_sel.T),
            "convdiag": cd,
            "w_x_T": np.ascontiguousarray(W_x[:, sl].T),
            "w_dt_T": np.ascontiguousarray(W_dt[sl].T),
            "b_dt_col": np.ascontiguousarray(b_dt[sl])[:, None],
            "a_col": np.ascontiguousarray(-np.exp(A_log[sl])),
            "d_col": np.ascontiguousarray(D[sl])[:, None],
            "conv_b_col": np.ascontiguousarray(conv_b[sl])[:, None],
            "w_out_T": np.ascontiguousarray(W_out[:, sl].T),
            "id128": id128,
        })
    return in_maps


_NC_CACHE = {}


def _get_nc(cfg):
    key = tuple(sorted(cfg.items()))
    if key not in _NC_CACHE:
        _NC_CACHE[key] = build_nc(cfg)
    return _NC_CACHE[key]


def run_cfg(inputs, cfg, time_iters=0):
    nc = _get_nc(cfg)
    NC = cfg["NC"]
    in_maps = prep_inputs(inputs, cfg)
    res = run_bass_kernel_spmd(nc, in_maps, list(range(NC)))
    wall_ns = None
    if time_iters:
        times = []
        for _ in range(time_iters):
            t0 = time.perf_counter()
            res = run_bass_kernel_spmd(nc, in_maps, list(range(NC)))
            times.append(time.perf_counter() - t0)
        wall_ns = min(times) * 1e9
    out = np.concatenate([res.results[c]["out"] for c in range(NC)], 0)
    B, L, DM = cfg["B"], cfg["L"], cfg["DM"]
    return out.reshape(B, L, DM), wall_ns


def kernel(**inputs):
    out, _ = run_cfg(inputs, CFG_FULL)
    return out.astype(np.float32)
